# revision 20
# baseline (speedup 1.0000x reference)
"""Causal self-attention (B=2, T=2048, C=1024, H=16) on 8 trn2 NeuronCores.

Sharding: core c handles batch b = c//4 and head-group g = c%4 (4 heads,
256 qkv channels each).  c_attn is column-split, c_proj is row-split
(Megatron style); each core emits a partial [T, C] projection output and
the host sums the 4 partials per batch (+ b_proj).  No device collectives.

Per-core kernel (all matmuls float32r = FP22 multiplies, fp32 accumulate):
  phase 1: Q^T,K^T  [256, T] = (x@Wq)^T via lhsT=W, rhs=x^T
           V'       [T, 4*65] = x@Wv (+ ones column per head for the
           softmax denominator)
  phase 2: per head pair, per 512-wide q chunk, per 128-wide k tile:
           S^T [128k, 2*512q] = K_h^T.T @ Q_h^T for both heads into one
           2-bank PSUM tile (two K=64 matmuls row-packed at partitions
           0/64), ONE exp on ScalarE (scale=1/8 folded in; safe without
           max-subtraction for N(0,1) scores), causal-mask multiply on
           diagonal k tiles only (single strided DVE op over both heads,
           partial width), O^T accumulation [65, 512] per head with the
           65th row = softmax denominator via V's fused ones column.
           normalize: DVE reciprocal of row 64, PE ones-matmul broadcast,
           ScalarE copy to SBUF, DVE multiply (+ v-bias) into y^T
  phase 3: partial = y^T.T @ Wproj_rows, DMA out
Phases are emitted interleaved across q chunks so exp/DVE work overlaps
phase-1/3 matmuls; initial loads are spread across 3 DMA queues.
"""

import numpy as np
from contextlib import ExitStack

B, T, C, NHEAD = 2, 2048, 1024, 16
HL = 4           # heads per core
HD = 64          # head dim
LQK = 512        # local q+k channels (2*HL*HD)
LV = 256         # local v channels
QC = 512         # q chunk width
NQ = T // QC     # 4 q chunks
NCC = C // 128   # 8 contraction chunks
NT = T // 128    # 16 row tiles
VW = 65          # V' width per head (64 + ones col)

_CACHE = {}


def _build_program(reps=1, mmdt="f32r"):
    import concourse.tile as tile
    from concourse import bacc, mybir

    F32 = mybir.dt.float32
    F32R = mybir.dt.float32r if mmdt == "f32r" else mybir.dt.bfloat16
    EXP = mybir.ActivationFunctionType.Exp

    nc = bacc.Bacc("TRN2", target_bir_lowering=False, debug=False, num_devices=8)

    xt = nc.dram_tensor("xt", [128, NCC, T], F32R, kind="ExternalInput").ap()
    wqk = nc.dram_tensor("wqk", [128, NCC, LQK], F32R, kind="ExternalInput").ap()
    wv = nc.dram_tensor("wv", [128, NCC, LV], F32R, kind="ExternalInput").ap()
    wp = nc.dram_tensor("wp", [128, 2, C], F32R, kind="ExternalInput").ap()
    bqk = nc.dram_tensor("bqk", [128, 4], F32, kind="ExternalInput").ap()
    bv = nc.dram_tensor("bv", [128, 2], F32, kind="ExternalInput").ap()
    mask = nc.dram_tensor("mask", [128, 2 * 896], F32R, kind="ExternalInput").ap()
    out = nc.dram_tensor("out", [T, C], F32, kind="ExternalOutput").ap()

    with (
        tile.TileContext(nc) as tc,
        ExitStack() as ctx,
        nc.allow_low_precision(reason="float32r storage is deliberate (FP22 matmuls)"),
    ):
        consts = ctx.enter_context(tc.tile_pool(name="consts", bufs=1))
        xpool = ctx.enter_context(tc.tile_pool(name="xp", bufs=2))
        qkpool = ctx.enter_context(tc.tile_pool(name="qk", bufs=8))
        vpool = ctx.enter_context(tc.tile_pool(name="v", bufs=16))
        ypool = ctx.enter_context(tc.tile_pool(name="y", bufs=4))
        ppool = ctx.enter_context(tc.tile_pool(name="pt", bufs=5))
        opool = ctx.enter_context(tc.tile_pool(name="obuf", bufs=3))
        rpool = ctx.enter_context(tc.tile_pool(name="rbc", bufs=2))
        ps1 = ctx.enter_context(tc.tile_pool(name="ps1", bufs=2, space="PSUM"))
        psS = ctx.enter_context(tc.tile_pool(name="psS", bufs=2, space="PSUM"))
        psO = ctx.enter_context(tc.tile_pool(name="psO", bufs=2, space="PSUM"))

        bqk_sb = consts.tile([128, 4], F32)
        nc.gpsimd.dma_start(out=bqk_sb, in_=bqk)
        bv_sb = consts.tile([128, 2], F32)
        nc.gpsimd.dma_start(out=bv_sb, in_=bv)
        wqk_sb = consts.tile([128, NCC, LQK], F32R)
        nc.gpsimd.dma_start(out=wqk_sb[:, 0:4, :], in_=wqk[:, 0:4, :])
        nc.scalar.dma_start(out=wqk_sb[:, 4:8, :], in_=wqk[:, 4:8, :])
        wv_sb = consts.tile([128, NCC, LV], F32R)
        nc.scalar.dma_start(out=wv_sb, in_=wv)
        wp_sb = consts.tile([128, 2, C], F32R)
        nc.scalar.dma_start(out=wp_sb, in_=wp)
        mask_sb = consts.tile([128, 2 * 896], F32R)
        nc.gpsimd.dma_start(out=mask_sb, in_=mask)
        mask3 = mask_sb.rearrange("p (r u) -> p r u", u=896)
        # all-ones block: mask columns >= 832 are 1.0 for every row
        ones_sb = mask_sb[:, 832:896]

        def emit_rep(R):
            # persistent activations, chunked for fine-grained deps
            qT = [qkpool.tile([128, 2, QC], F32R, tag="qT", name=f"qT{R}_{j}")
                  for j in range(NQ)]
            kT = [qkpool.tile([128, 2, QC], F32R, tag="kT", name=f"kT{R}_{j}")
                  for j in range(NQ)]
            vS = [vpool.tile([128, HL * VW], F32R, tag="vS", name=f"vS{R}_{t}")
                  for t in range(NT)]
            yT = [ypool.tile([128, 2, QC], F32R, tag="yT", name=f"yT{R}_{j}")
                  for j in range(NQ)]

            # ---------------- phase 1: qkv projections ----------------
            def emit_ph1(j):
                xp = xpool.tile([128, NCC, QC], F32R, tag="xp", name=f"xp{R}_{j}")
                if j == 0:
                    nc.sync.dma_start(out=xp[:, 0:4, :], in_=xt[:, 0:4, 0:QC])
                    nc.gpsimd.dma_start(out=xp[:, 4:8, :], in_=xt[:, 4:8, 0:QC])
                else:
                    nc.sync.dma_start(out=xp, in_=xt[:, :, QC * j : QC * (j + 1)])
                # Q^T / K^T channel tiles (m: Q0 Q1 K0 K1)
                for m in range(4):
                    ps = ps1.tile([128, QC], F32, tag="ps1")
                    for c in range(NCC):
                        nc.tensor.matmul(
                            ps,
                            lhsT=wqk_sb[:, c, 128 * m : 128 * (m + 1)],
                            rhs=xp[:, c, :],
                            start=(c == 0),
                            stop=(c == NCC - 1),
                        )
                    dst = (qT if m < 2 else kT)[j][:, m % 2, :]
                    nc.vector.tensor_scalar_add(dst, ps, bqk_sb[:, m : m + 1])
                # V row tiles
                for t4 in range(4):
                    tt = 4 * j + t4
                    ps = ps1.tile([128, QC], F32, tag="ps1")
                    psv = ps[:, 0:LV]
                    for c in range(NCC):
                        nc.tensor.matmul(
                            psv,
                            lhsT=xp[:, c, 128 * t4 : 128 * (t4 + 1)],
                            rhs=wv_sb[:, c, :],
                            start=(c == 0),
                            stop=(c == NCC - 1),
                        )
                    vst = vS[tt].rearrange("p (h e) -> p h e", e=VW)
                    nc.vector.tensor_copy(
                        vst[:, :, 0:HD],
                        psv.rearrange("p (h e) -> p h e", e=HD),
                    )
                    nc.vector.tensor_copy(
                        vst[:, :, HD : HD + 1],
                        mask_sb[:, 832 : 832 + HL].rearrange("p (h e) -> p h e", e=1),
                    )

            # ---------------- phase 2: causal attention ----------------
            def emit_ph2(j):
                for pair in range(2):  # heads (2*pair, 2*pair+1)
                    nk = 4 * j + 4
                    oth = [
                        psO.tile([128, QC], F32, tag="psO",
                                 name=f"ot{R}_{j}_{pair}_{hh}")
                        for hh in range(2)
                    ]
                    pts = [None] * nk

                    def emit_ot(ki):
                        for hh in range(2):
                            h = 2 * pair + hh
                            nc.tensor.matmul(
                                oth[hh][0:VW, :],
                                lhsT=vS[ki][:, VW * h : VW * (h + 1)],
                                rhs=pts[ki][:, QC * hh : QC * (hh + 1)],
                                start=(ki == 0),
                                stop=(ki == nk - 1),
                            )

                    for ki in range(nk):
                        # both heads' S^T into one 2-bank psum tile
                        sps = psS.tile([128, 2 * QC], F32, tag="psS",
                                       name=f"sps{R}_{j}_{pair}_{ki}")
                        for hh in range(2):
                            bp = 64 * hh
                            nc.tensor.matmul(
                                sps[:, QC * hh : QC * (hh + 1)],
                                lhsT=kT[ki // 4][bp : bp + 64, pair,
                                                 128 * (ki % 4) : 128 * (ki % 4 + 1)],
                                rhs=qT[j][bp : bp + 64, pair, :],
                                start=True,
                                stop=True,
                            )
                        pt = ppool.tile([128, 2 * QC], F32R, tag="pt",
                                        name=f"pt{R}_{j}_{pair}_{ki}")
                        nc.scalar.activation(pt, sps, EXP, scale=0.125)
                        pts[ki] = pt
                        if ki >= 4 * j:
                            # diagonal tile: causal mask (one strided op over
                            # both heads; only first d+128 cols can be masked)
                            d = 128 * (ki - 4 * j)
                            s, w = 384 - d, d + 128
                            pt3 = pt.rearrange("p (r q) -> p r q", q=QC)
                            nc.vector.tensor_mul(
                                pt3[:, :, 0:w], pt3[:, :, 0:w],
                                mask3[:, :, s : s + w],
                            )
                        # software pipeline: PE runs S(ki) two steps ahead of OT
                        if ki >= 2:
                            emit_ot(ki - 2)
                    emit_ot(nk - 2)
                    emit_ot(nk - 1)

                    # normalize + v-bias, write y^T
                    rcs = [
                        rpool.tile([128, QC], F32R, tag="rc",
                                   name=f"rc{R}_{j}_{pair}_{hh}")
                        for hh in range(2)
                    ]
                    rbc = rpool.tile([128, QC], F32, tag="rbc",
                                     name=f"rbc{R}_{j}_{pair}")
                    for hh in range(2):
                        nc.vector.reciprocal(rcs[hh][64:65, :], oth[hh][64:65, :])
                        # broadcast recip row to 64 partitions: K=1 ones matmul
                        rps = ps1.tile([128, QC], F32, tag="ps1",
                                       name=f"rps{R}_{j}_{pair}_{hh}")
                        nc.tensor.matmul(
                            rps[0:64, :],
                            lhsT=ones_sb[64:65, :],
                            rhs=rcs[hh][64:65, :],
                            start=True,
                            stop=True,
                        )
                        nc.scalar.copy(rbc[64 * hh : 64 * hh + 64, :], rps[0:64, :])
                    for hh in range(2):
                        ydst = yT[j][64 * hh : 64 * hh + 64, pair, :]
                        nc.vector.tensor_mul(
                            ydst, oth[hh][0:64, :], rbc[64 * hh : 64 * hh + 64, :]
                        )
                        nc.vector.tensor_scalar_add(
                            ydst, ydst,
                            bv_sb[64 * hh : 64 * hh + 64, pair : pair + 1],
                        )

            # ---------------- phase 3: output projection ----------------
            def emit_ph3(j):
                for tt in range(4 * j, 4 * j + 4):
                    for n in range(2):
                        ps = ps1.tile([128, QC], F32, tag="ps1")
                        for c2 in range(2):
                            nc.tensor.matmul(
                                ps,
                                lhsT=yT[j][:, c2,
                                           128 * (tt % 4) : 128 * (tt % 4 + 1)],
                                rhs=wp_sb[:, c2, QC * n : QC * (n + 1)],
                                start=(c2 == 0),
                                stop=(c2 == 1),
                            )
                        ob = opool.tile([128, QC], F32, tag="obuf")
                        nc.vector.tensor_copy(ob, ps)
                        eng = nc.gpsimd if (tt + n) % 2 == 0 else nc.sync
                        eng.dma_start(
                            out=out[128 * tt : 128 * (tt + 1),
                                    QC * n : QC * (n + 1)],
                            in_=ob,
                        )

            # ---------------- interleaved emission ----------------
            emit_ph1(0)
            emit_ph1(1)
            emit_ph2(0)
            emit_ph1(2)
            emit_ph3(0)
            emit_ph2(1)
            emit_ph1(3)
            emit_ph3(1)
            emit_ph2(2)
            emit_ph3(2)
            emit_ph2(3)
            emit_ph3(3)

        for r in range(reps):
            emit_rep(r)

    nc.compile()
    return nc


def _host_inputs(x, w_attn, b_attn, w_proj, core, mmdt="f32r"):
    """Per-core input arrays, pre-laid-out for the kernel."""
    if mmdt == "f32r":
        mdt = np.float32
    else:
        import ml_dtypes
        mdt = ml_dtypes.bfloat16
    b, g = core // 4, core % 4
    q0, k0, v0 = g * 256, C + g * 256, 2 * C + g * 256

    xtc = np.ascontiguousarray(
        x[b].T.reshape(NCC, 128, T).transpose(1, 0, 2)
    )  # [128, 8, 2048]
    wqk_cols = np.concatenate(
        [w_attn[:, q0 : q0 + 256], w_attn[:, k0 : k0 + 256]], axis=1
    )  # [1024, 512]
    wqkc = np.ascontiguousarray(wqk_cols.reshape(NCC, 128, LQK).transpose(1, 0, 2))
    wvc = np.ascontiguousarray(
        w_attn[:, v0 : v0 + 256].reshape(NCC, 128, LV).transpose(1, 0, 2)
    )
    wpc = np.ascontiguousarray(
        w_proj[g * 256 : (g + 1) * 256, :].reshape(2, 128, C).transpose(1, 0, 2)
    )
    bqkc = np.ascontiguousarray(
        np.stack(
            [
                b_attn[q0 : q0 + 128],
                b_attn[q0 + 128 : q0 + 256],
                b_attn[k0 : k0 + 128],
                b_attn[k0 + 128 : k0 + 256],
            ],
            axis=1,
        )
    )  # [128, 4]
    bvc = np.ascontiguousarray(
        np.stack([b_attn[v0 : v0 + 128], b_attn[v0 + 128 : v0 + 256]], axis=1)
    )  # [128, 2]
    ku = np.arange(128)[:, None]
    uu = np.arange(896)[None, :]
    maskc = (uu >= ku + 384).astype(np.float32)  # [128, 896]
    maskc = np.ascontiguousarray(np.concatenate([maskc, maskc], axis=1))
    return {
        "xt": xtc.astype(mdt),
        "wqk": wqkc.astype(mdt),
        "wv": wvc.astype(mdt),
        "wp": wpc.astype(mdt),
        "bqk": bqkc.astype(np.float32),
        "bv": bvc.astype(np.float32),
        "mask": maskc.astype(mdt),
    }


def _get_program(reps=1, mmdt="f32r"):
    key = ("nc", reps, mmdt)
    if key not in _CACHE:
        _CACHE[key] = _build_program(reps, mmdt)
    return _CACHE[key]


def kernel(x, w_attn, b_attn, w_proj, b_proj):
    from concourse.bass_utils import run_bass_kernel_spmd

    x = np.asarray(x, np.float32)
    w_attn = np.asarray(w_attn, np.float32)
    b_attn = np.asarray(b_attn, np.float32)
    w_proj = np.asarray(w_proj, np.float32)
    b_proj = np.asarray(b_proj, np.float32)

    nc = _get_program()
    in_maps = [_host_inputs(x, w_attn, b_attn, w_proj, c) for c in range(8)]
    res = run_bass_kernel_spmd(nc, in_maps, core_ids=list(range(8)))
    partials = [res.results[c]["out"] for c in range(8)]
    out = np.empty((B, T, C), np.float32)
    for b in range(B):
        acc = np.sum(
            np.stack(partials[4 * b : 4 * b + 4]).astype(np.float64), axis=0
        )
        out[b] = (acc + b_proj.astype(np.float64)).astype(np.float32)
    return out
